# revision 2
# baseline (speedup 1.0000x reference)
"""Trainium2 Bass kernel for e3nn-style BatchNorm (instance norm over graphs).

v3: static per-graph slots with per-position sizes + engine rebalancing.

  Graphs are sorted by row count (descending) and assigned rank r ->
  (core r%8, slot position r//8), so slot position k can be compiled with
  its own row count S_k = 128*R_k = the max over cores at that position.

  Scalar block (cols 0:128) ships TRANSPOSED f32 [128 chan, rows]:
  per-graph scale/shift are per-PARTITION scalars -> stats are two ACT
  accumulator scans (Square, Copy) and the apply is one fused ACT
  affine (Identity with [128,1] scale/bias APs).  All-f32 -> abs err
  ~1e-7 (the rel metric has a 1e-3 denominator floor).

  Vector block (cols 128:480) is row-major fp16 [p, r, 352]; square on
  DVE (2x fp16) with a few trailing r-slices on GpSimd; per-channel
  sums via R accumulating ones-matmuls (PE); per-graph params
  broadcast by a K=1 ones-matmul and d-expanded from PSUM by DVE
  copies; apply is a DVE/GpSimd fp16 multiply with an R-stride-0
  broadcast AP, in place, stored straight out.

  Software pipeline with lookahead so each engine always has
  independent next-slot work queued ahead of dependent current-slot
  work.  DMA queues: xv loads on SP, xs loads + os stores on
  Pool(SWDGE), ov stores on ACT.
"""

import sys

if "/opt/trn_rl_repo" not in sys.path:
    sys.path.insert(0, "/opt/trn_rl_repo")

import numpy as np

P = 128          # partitions
NCORES = 8
G = 64           # total graphs
GPC = G // NCORES  # slots per core
CS = 128         # scalar channels
CV = 352         # vector-block columns (64x3 + 32x5)
EPS = 1e-5
ACT_SQ = 2       # trailing r-slices of the square done on ACT

_prog_cache = {}


def _build(r_list):
    """One SPMD program for slot sizes S_k = 128*r_list[k]."""
    import concourse.bacc as bacc
    import concourse.bass as bass
    import concourse.tile as tile
    from concourse import mybir

    f32 = mybir.dt.float32
    f16 = mybir.dt.float16
    Alu = mybir.AluOpType
    Act = mybir.ActivationFunctionType

    soff = np.concatenate([[0], np.cumsum([P * r for r in r_list])])
    rows = int(soff[-1])

    nc = bacc.Bacc("TRN2", target_bir_lowering=False, debug=False,
                   num_devices=NCORES)

    nc.t_xv = nc.dram_tensor("xv", [rows, CV], f16, kind="ExternalInput")
    nc.t_xs = nc.dram_tensor("xst", [P, rows], f32, kind="ExternalInput")
    nc.t_wst = nc.dram_tensor("wst", [P, 1], f32, kind="ExternalInput")
    nc.t_bst = nc.dram_tensor("bst", [P, 1], f32, kind="ExternalInput")
    nc.t_invc = nc.dram_tensor("invc", [P, GPC], f32, kind="ExternalInput")
    nc.t_icv = nc.dram_tensor("icv", [1, GPC], f32, kind="ExternalInput")
    nc.t_dinv = nc.dram_tensor("dinv", [1, 96], f32, kind="ExternalInput")
    nc.t_wv = nc.dram_tensor("wv", [1, 96], f32, kind="ExternalInput")
    nc.t_ov = nc.dram_tensor("ov", [rows, CV], f16, kind="ExternalOutput")
    nc.t_os = nc.dram_tensor("ost", [P, rows], f16, kind="ExternalOutput")

    def xv_ap(t, g, r0, r1):
        R = r_list[g]
        return bass.AP(tensor=t, offset=(int(soff[g]) + r0) * CV,
                       ap=[[R * CV, P], [CV, r1 - r0], [1, CV]])

    def xs_ap(t, g):
        S = P * r_list[g]
        return bass.AP(tensor=t, offset=int(soff[g]),
                       ap=[[rows, P], [1, S]])

    with tile.TileContext(nc) as tc:
        with (
            tc.tile_pool(name="const", bufs=1) as cp,
            tc.tile_pool(name="xv", bufs=5) as xvp,
            tc.tile_pool(name="xs", bufs=4) as xsp,
            tc.tile_pool(name="sq", bufs=3) as sqp,
            tc.tile_pool(name="os", bufs=3) as osp,
            tc.tile_pool(name="par", bufs=3) as pp,
            tc.tile_pool(name="ps", bufs=2, space="PSUM") as psp,
            tc.tile_pool(name="bc", bufs=2, space="PSUM") as bcp,
        ):
            # constants
            ones128 = cp.tile([P, 1], f16, tag="ones128")
            nc.vector.memset(ones128[:], 1.0)
            ones1 = cp.tile([1, P], f16, tag="ones1")
            nc.vector.memset(ones1[:], 1.0)
            w_st = cp.tile([P, 1], f32, tag="w_st")
            nc.gpsimd.dma_start(out=w_st[:], in_=nc.t_wst.ap())
            b_st = cp.tile([P, 1], f32, tag="b_st")
            nc.gpsimd.dma_start(out=b_st[:], in_=nc.t_bst.ap())
            invc = cp.tile([P, GPC], f32, tag="invc")
            nc.gpsimd.dma_start(out=invc[:], in_=nc.t_invc.ap())
            icv = cp.tile([1, GPC], f32, tag="icv")
            nc.gpsimd.dma_start(out=icv[:], in_=nc.t_icv.ap())
            dinv = cp.tile([1, 96], f32, tag="dinv")
            nc.gpsimd.dma_start(out=dinv[:], in_=nc.t_dinv.ap())
            w_v = cp.tile([1, 96], f32, tag="w_v")
            nc.gpsimd.dma_start(out=w_v[:], in_=nc.t_wv.ap())
            eps_s = cp.tile([P, 1], f32, tag="eps_s")
            nc.vector.memset(eps_s[:], EPS)
            eps_v = cp.tile([1, 1], f32, tag="eps_v")
            nc.vector.memset(eps_v[:], EPS)

            st = [dict() for _ in range(GPC)]

            def emit_load_xv(g, first=False):
                R = r_list[g]
                xv = xvp.tile([P, R, CV], f16, tag="xv")
                if first:
                    # split the first load across two queues to cut the
                    # pipeline fill latency
                    h = R // 2
                    nc.sync.dma_start(out=xv[:, 0:h, :],
                                      in_=xv_ap(nc.t_xv, g, 0, h))
                    nc.scalar.dma_start(out=xv[:, h:R, :],
                                        in_=xv_ap(nc.t_xv, g, h, R))
                else:
                    nc.sync.dma_start(out=xv[:], in_=xv_ap(nc.t_xv, g, 0, R))
                st[g]["xv"] = xv

            def emit_load_xs(g):
                R = r_list[g]
                xs = xsp.tile([P, P * R], f32, tag="xs")
                nc.gpsimd.dma_start(out=xs[:], in_=xs_ap(nc.t_xs, g))
                st[g]["xs"] = xs

            def emit_sqv(g):
                # two half-size tiles: smaller SBUF footprint, finer overlap
                R = r_list[g]
                xv = st[g]["xv"]
                k = R - ACT_SQ
                h = k // 2
                sq1 = sqp.tile([P, h, CV], f16, tag="sqv")
                nc.vector.tensor_tensor(out=sq1[:, 0:h, :], in0=xv[:, 0:h, :],
                                        in1=xv[:, 0:h, :], op=Alu.mult)
                sq2 = sqp.tile([P, R - h, CV], f16, tag="sqv")
                nc.vector.tensor_tensor(out=sq2[:, 0:k - h, :],
                                        in0=xv[:, h:k, :],
                                        in1=xv[:, h:k, :], op=Alu.mult)
                st[g].update(sq1=sq1, sq2=sq2, sqh=h, sqk=k)

            def emit_scans(g):
                R = r_list[g]
                xv = st[g]["xv"]
                sq2 = st[g]["sq2"]
                h, k = st[g]["sqh"], st[g]["sqk"]
                nc.scalar.activation(out=sq2[:, k - h:k - h + ACT_SQ, :],
                                     in_=xv[:, k:R, :], func=Act.Square)
                xs = st[g]["xs"]
                scr = osp.tile([P, P * R], f16, tag="scr")
                sx2 = pp.tile([P, 1], f32, tag="sx2", name="sx2")
                nc.scalar.activation(out=scr[:], in_=xs[:], func=Act.Square,
                                     accum_out=sx2[:])
                sx = pp.tile([P, 1], f32, tag="sx", name="sx")
                nc.scalar.activation(out=scr[:], in_=xs[:], func=Act.Copy,
                                     accum_out=sx[:])
                st[g].update(scr=scr, sx2=sx2, sx=sx)

            def emit_mm(g):
                R = r_list[g]
                sq1, sq2 = st[g]["sq1"], st[g]["sq2"]
                h = st[g]["sqh"]
                ps = psp.tile([1, CV], f32, tag="ps")
                for r in range(R):
                    rhs = sq1[:, r, :] if r < h else sq2[:, r - h, :]
                    nc.tensor.matmul(out=ps[:], lhsT=ones128[:], rhs=rhs,
                                     start=(r == 0), stop=(r == R - 1))
                st[g]["ps"] = ps

            def emit_params(g):
                ps, sx2, sx = st[g]["ps"], st[g]["sx2"], st[g]["sx"]
                t = lambda shape, name: pp.tile(shape, f32, tag=name, name=name)
                iv_g = invc[:, g:g + 1]
                # scalar params: A = w/sqrt(var+eps), B = b - m*A   [128,1]
                m = t([P, 1], "m")
                nc.vector.tensor_scalar_mul(out=m[:], in0=sx[:], scalar1=iv_g)
                m2 = t([P, 1], "m2")
                nc.vector.tensor_tensor(out=m2[:], in0=m[:], in1=m[:],
                                        op=Alu.mult)
                var = t([P, 1], "var")
                nc.vector.scalar_tensor_tensor(out=var[:], in0=sx2[:],
                                               scalar=iv_g, in1=m2[:],
                                               op0=Alu.mult, op1=Alu.subtract)
                sd = t([P, 1], "sd")
                nc.scalar.activation(out=sd[:], in_=var[:], func=Act.Sqrt,
                                     bias=eps_s[:], scale=1.0)
                a_s = t([P, 1], "a_s")
                nc.vector.reciprocal(out=a_s[:], in_=sd[:])
                nc.vector.tensor_tensor(out=a_s[:], in0=a_s[:], in1=w_st[:],
                                        op=Alu.mult)
                b_s = t([P, 1], "b_s")
                nc.vector.tensor_tensor(out=b_s[:], in0=m[:], in1=a_s[:],
                                        op=Alu.mult)
                nc.vector.tensor_tensor(out=b_s[:], in0=b_st[:], in1=b_s[:],
                                        op=Alu.subtract)

                # vector params: fn = (sum x^2)*invc/d; A = w/sqrt(fn+eps)
                e = pp.tile([1, 96], f32, tag="e", name="e")
                nc.vector.tensor_reduce(
                    out=e[:, 0:64],
                    in_=ps[:, 0:192].rearrange("p (j d) -> p j d", d=3),
                    axis=mybir.AxisListType.X, op=Alu.add)
                nc.vector.tensor_reduce(
                    out=e[:, 64:96],
                    in_=ps[:, 192:352].rearrange("p (j d) -> p j d", d=5),
                    axis=mybir.AxisListType.X, op=Alu.add)
                fn = pp.tile([1, 96], f32, tag="fn", name="fn")
                nc.vector.tensor_scalar_mul(out=fn[:], in0=e[:],
                                            scalar1=icv[:, g:g + 1])
                nc.vector.tensor_tensor(out=fn[:], in0=fn[:], in1=dinv[:],
                                        op=Alu.mult)
                nc.scalar.activation(out=fn[:], in_=fn[:], func=Act.Sqrt,
                                     bias=eps_v[:], scale=1.0)
                pv = pp.tile([1, 96], f32, tag="pv", name="pv")
                nc.vector.reciprocal(out=pv[:], in_=fn[:])
                nc.vector.tensor_tensor(out=pv[:], in0=pv[:], in1=w_v[:],
                                        op=Alu.mult)
                pv16 = pp.tile([1, 96], f16, tag="pv16", name="pv16")
                nc.vector.tensor_copy(out=pv16[:], in_=pv[:])
                bc = bcp.tile([P, 96], f32, tag="bc")
                nc.tensor.matmul(out=bc[:], lhsT=ones1[:], rhs=pv16[:],
                                 start=True, stop=True)
                # d-expand PSUM -> packed fp16 [128, 352] (DVE copies)
                pex = pp.tile([P, CV], f16, tag="pex")
                bca = bc[:]
                nc.vector.tensor_copy(
                    out=pex[:, 0:192].rearrange("p (j d) -> p j d", d=3),
                    in_=bass.AP(tensor=bca.tensor, offset=bca.offset,
                                ap=[bca.ap[0], [1, 64], [0, 3]]))
                nc.vector.tensor_copy(
                    out=pex[:, 192:352].rearrange("p (j d) -> p j d", d=5),
                    in_=bass.AP(tensor=bca.tensor, offset=bca.offset + 64,
                                ap=[bca.ap[0], [1, 32], [0, 5]]))
                st[g].update(a_s=a_s, b_s=b_s, pex=pex)

            def emit_apply(g):
                R = r_list[g]
                xv, xs, scr = st[g]["xv"], st[g]["xs"], st[g]["scr"]
                pex = st[g]["pex"]
                pexa = pex[:]

                def bcast(r0, r1):
                    return bass.AP(tensor=pexa.tensor, offset=pexa.offset,
                                   ap=[pexa.ap[0], [0, r1 - r0], pexa.ap[1]])

                nc.vector.tensor_tensor(out=xv[:], in0=xv[:],
                                        in1=bcast(0, R), op=Alu.mult)
                nc.scalar.dma_start(out=xv_ap(nc.t_ov, g, 0, R), in_=xv[:])
                nc.scalar.activation(out=scr[:], in_=xs[:], func=Act.Identity,
                                     scale=st[g]["a_s"][:], bias=st[g]["b_s"][:])
                nc.gpsimd.dma_start(out=xs_ap(nc.t_os, g), in_=scr[:])

            # software pipeline; process the smallest slot first for a
            # fast fill, then the rest largest-to-smallest.  Deep staggered
            # load lookahead soaks up DMA bandwidth before stores begin.
            seq = [GPC - 1] + list(range(GPC - 1))
            emit_load_xv(seq[0], first=True)
            emit_load_xs(seq[0])
            emit_load_xv(seq[1])
            emit_load_xs(seq[1])
            emit_sqv(seq[0])
            emit_load_xv(seq[2])
            emit_load_xs(seq[2])
            emit_scans(seq[0])
            emit_mm(seq[0])
            emit_load_xv(seq[3])
            for i in range(GPC):
                if i + 4 < GPC:
                    emit_load_xv(seq[i + 4])
                if i + 3 < GPC:
                    emit_load_xs(seq[i + 3])
                emit_params(seq[i])
                if i + 1 < GPC:
                    emit_sqv(seq[i + 1])
                    emit_scans(seq[i + 1])
                emit_apply(seq[i])
                if i + 1 < GPC:
                    emit_mm(seq[i + 1])

    nc.compile()
    return nc


def kernel(input, batch_id_tensor, weight, bias, _trace=False):
    from concourse import bass_utils

    x = np.asarray(input, dtype=np.float32)
    bid = np.asarray(batch_id_tensor).astype(np.int64)
    w = np.asarray(weight, dtype=np.float32)
    b = np.asarray(bias, dtype=np.float32)
    n = x.shape[0]

    cnt = np.bincount(bid, minlength=G).astype(np.int64)
    edges = np.zeros(G + 1, dtype=np.int64)
    np.cumsum(cnt, out=edges[1:])

    # sort graphs by size desc: rank j -> (core j%8, position j//8)
    perm = np.argsort(-cnt, kind="stable")
    r_list = tuple(max(1, int(-(-cnt[perm[8 * k]] // P)))
                   for k in range(GPC))
    soff = np.concatenate([[0], np.cumsum([P * r for r in r_list])])
    rows = int(soff[-1])

    if r_list not in _prog_cache:
        _prog_cache[r_list] = _build(r_list)
    nc = _prog_cache[r_list]

    cntc = np.maximum(cnt, 1).astype(np.float32)
    dinv = np.concatenate([np.full(64, 1 / 3.0, np.float32),
                           np.full(32, 1 / 5.0, np.float32)])
    in_maps = []
    for c in range(NCORES):
        xv = np.zeros((rows, CV), dtype=np.float16)
        xst = np.zeros((P, rows), dtype=np.float32)
        ic = np.empty(GPC, np.float32)
        for g in range(GPC):
            gg = int(perm[8 * g + c])
            lo, hi = int(edges[gg]), int(edges[gg + 1])
            k = hi - lo
            o = int(soff[g])
            if k:
                xv[o:o + k] = x[lo:hi, CS:]
                xst[:, o:o + k] = x[lo:hi, 0:CS].T
            ic[g] = 1.0 / cntc[gg]
        in_maps.append({
            "xv": xv,
            "xst": xst,
            "wst": np.ascontiguousarray(w[0:CS, None]),
            "bst": np.ascontiguousarray(b[:, None]),
            "invc": np.broadcast_to(ic, (P, GPC)).copy(),
            "icv": np.ascontiguousarray(ic[None, :]),
            "dinv": np.ascontiguousarray(dinv[None, :]),
            "wv": np.ascontiguousarray(w[None, CS:]),
        })

    res = bass_utils.run_bass_kernel_spmd(
        nc, in_maps, core_ids=list(range(NCORES)), trace=_trace)

    out = np.empty((n, 480), dtype=np.float32)
    for c in range(NCORES):
        ov = res.results[c]["ov"]
        ost = res.results[c]["ost"]
        for g in range(GPC):
            gg = int(perm[8 * g + c])
            lo, hi = int(edges[gg]), int(edges[gg + 1])
            k = hi - lo
            o = int(soff[g])
            if k:
                out[lo:hi, CS:] = ov[o:o + k]
                out[lo:hi, 0:CS] = ost[:, o:o + k].T
    if _trace:
        return out, res
    return out
